# revision 23
# baseline (speedup 1.0000x reference)
"""Trainium2 Bass kernel v9: 2D parallel-beam forward projection (Radon).

Input:  x [2, 256, 256, 1] float32
Output: sinogram [2, 180, 363, 1] float32

Strategy (8 NeuronCores, SPMD), v9 = DMA-descriptor gather:
  - Angles interleaved across cores (core c: angles c::8). Per core, the
    ~8167 rays are dealt (longest-first round-robin) onto 128 partition
    lanes; each lane owns whole rays.
  - Cell = (16-row window u, 7-px x-window xq) of the (possibly
    transposed) image, as in v7: per-angle transpose trick keeps the
    ray's y-step >= 0.707, so a ray has ~20 cells.
  - Pixels: one HBM table gtbl[1184, 256] f16; row (ori, u, xq) holds
    the cell's 2 batches x 8 w-rows x (2h x 8j) pixels. nc.gpsimd.
    dma_gather streams cells: one 512B DMA descriptor per cell, index
    i -> partition i%128, so each lane receives its own cells in order.
    Descriptor generation is ~0.34ns/cell on GPSIMD and the copies run
    on the 16 DMA engines - the Q7 ap_gather bottleneck (34ns/idx) of
    v7/v8 is gone.
  - Weights (geometry-only, cached across calls): 128 f16 per cell
    (w x 2h x 8j), applied to both batches by two DVE muls.
  - DVE per chunk: 2 muls (fp16 2x), 7-level pairwise add tree (fp16
    2x) folding 128 -> 1 per (cell, batch), then one segmented
    tensor_reduce folding each ray's Lk cells into acc[128, 2*NR].
"""
import os
import sys
from contextlib import ExitStack

import numpy as np

for p in ("/opt/trn_rl_repo", "/root/.axon_site/_ro/trn_rl_repo"):
    if os.path.isdir(p) and p not in sys.path:
        sys.path.insert(0, p)

import concourse.bass as bass  # noqa: E402,F401
import concourse.bacc as bacc  # noqa: E402
import concourse.mybir as mybir  # noqa: E402
import concourse.tile as tile  # noqa: E402
from concourse import bass_utils  # noqa: E402

F32 = mybir.dt.float32
F16 = mybir.dt.float16
I16 = mybir.dt.int16

# ---- geometry constants (mirror of the reference) ----
VOL = 256
N_ANGLES = 180
N_DET = 363
N_SAMPLES = 363
CEN = (VOL - 1) / 2.0
DCEN = (N_DET - 1) / 2.0
SCEN = (N_SAMPLES - 1) / 2.0

N_U = 16          # 16-row windows
XW = 7            # x-slots covered per cell (entry spans XW+1 pixels)
N_XQ = 37         # ceil(257/7)
HALF = N_U * N_XQ             # 592 entries per orientation
NUM_ELEMS = 2 * HALF          # 1184 table rows
CSL = 128                     # weight slots per cell (8w x 2h x 8j)
ESL = 256                     # table row length in f16 (2 batches x CSL)

N_CORES = 8
N_LANES = 128
TARGET_C = 48    # cell-columns per chunk (<= MAX_C)
MAX_C = 48

_plan_cache = {}
_compile_cache = {}


def _f32(v):
    return np.float32(v)


def _ray_cells(k):
    """Merged cell decomposition for angle index k.

    Returns a list over detectors d of (cellidx[int], W[8, 16, ncell]); W
    slot 8*h + j (h = y-half) weights pixel img[16u + 8h + (w), 7*xq+j].
    """
    theta = _f32(k) * _f32(np.pi / N_ANGLES)
    c = np.cos(theta, dtype=np.float32)
    s = np.sin(theta, dtype=np.float32)
    flip = abs(s) > abs(c)
    ce, se = (s, c) if flip else (c, s)
    d = np.arange(N_DET, dtype=np.float32)[:, None] - _f32(DCEN)
    t = np.arange(N_SAMPLES, dtype=np.float32)[None, :] - _f32(SCEN)
    fx = (ce * d - se * t + _f32(CEN)).astype(np.float32)
    fy = (se * d + ce * t + _f32(CEN)).astype(np.float32)
    x0 = np.floor(fx).astype(np.int64)
    y0 = np.floor(fy).astype(np.int64)
    wx = (fx - x0).astype(np.float64)
    wy = (fy - y0).astype(np.float64)

    x_dead = (x0 < -1) | (x0 > 255)
    xslot = np.clip(x0, 0, 256)
    a0 = np.where((x0 >= 0) & (x0 < VOL), 1.0 - wx, 0.0)
    a1 = np.where((x0 + 1 >= 0) & (x0 + 1 < VOL), wx, 0.0)
    neg1 = x0 == -1
    a0 = np.where(neg1, wx, a0)
    a1 = np.where(neg1, 0.0, a1)

    lo = (~x_dead) & (y0 >= 0) & (y0 < VOL)
    hi = (~x_dead) & (y0 + 1 >= 0) & (y0 + 1 < VOL)

    D_idx = np.broadcast_to(np.arange(N_DET)[:, None], fx.shape)
    d_all = np.concatenate([D_idx[lo], D_idx[hi]])
    y_all = np.concatenate([y0[lo], y0[hi] + 1])
    tw_all = np.concatenate([(1.0 - wy)[lo], wy[hi]])
    a0_all = np.concatenate([a0[lo], a0[hi]])
    a1_all = np.concatenate([a1[lo], a1[hi]])
    xs_all = np.concatenate([xslot[lo], xslot[hi]])

    base = HALF if flip else 0
    u = y_all >> 4
    w = y_all & 7
    h = (y_all >> 3) & 1
    xq = xs_all // XW
    j0 = xs_all - xq * XW
    gkey = (d_all * N_U + u) * N_XQ + xq
    cells, inv = np.unique(gkey, return_inverse=True)
    ncell = len(cells)
    W = np.zeros((8, 16, ncell))
    np.add.at(W, (w, 8 * h + j0, inv), tw_all * a0_all)
    np.add.at(W, (w, 8 * h + j0 + 1, inv), tw_all * a1_all)
    cell_d = cells // (N_U * N_XQ)
    cell_e = cells % (N_U * N_XQ) + base
    bounds = np.searchsorted(cell_d, np.arange(N_DET + 1))
    out = []
    for dd in range(N_DET):
        i0, i1 = bounds[dd], bounds[dd + 1]
        out.append((cell_e[i0:i1], W[:, :, i0:i1]))
    return out


def _build_plan():
    """Geometry-only precompute shared across calls (input-independent)."""
    if "plan" in _plan_cache:
        return _plan_cache["plan"]
    rays = {}
    cnts = np.zeros((N_ANGLES, N_DET), dtype=np.int64)
    for k in range(N_ANGLES):
        rc = _ray_cells(k)
        rays[k] = rc
        cnts[k] = [len(r[0]) for r in rc]

    cores = []
    NRmax = 0
    lane_rays_all = []
    for c in range(N_CORES):
        ids = list(range(c, N_ANGLES, N_CORES))
        A, D = np.meshgrid(ids, np.arange(N_DET), indexing="ij")
        A, D = A.ravel(), D.ravel()
        lens = cnts[A, D]
        o = np.argsort(-lens, kind="stable")
        A, D, lens = A[o], D[o], lens[o]
        # snake-deal onto lanes to even out the sorted-length profile
        nround = (len(A) + N_LANES - 1) // N_LANES
        lane_idx = [[] for _ in range(N_LANES)]
        for r in range(nround):
            seg = range(r * N_LANES, min((r + 1) * N_LANES, len(A)))
            for i, gi in enumerate(seg):
                pl = i if r % 2 == 0 else (len(seg) - 1 - i)
                lane_idx[pl].append(gi)
        lane_rays = [(A[ix], D[ix], lens[ix])
                     for ix in (np.array(ix_, dtype=np.int64)
                                for ix_ in lane_idx)]
        lane_rays_all.append(lane_rays)
        NRmax = max(NRmax, max(len(a) for a, _, _ in lane_rays))

    # global sorted length profile (max across all lanes of all cores)
    P = np.zeros(NRmax, dtype=np.int64)
    for lane_rays in lane_rays_all:
        for A, D, lens in lane_rays:
            P[:len(lens)] = np.maximum(P[:len(lens)], lens)
    nz = int(np.count_nonzero(P))
    NR = nz

    # chunk schedule over ray positions: chunk = CHk rays x Lk cells.
    # Chunk column extents are padded to multiples of 4 so quad descriptors
    # align; rays themselves stay exact (quads may span ray boundaries).
    chunks = []   # (p0, CHk, Lk, coloff)
    coloff = 0
    p = 0
    while p < nz:
        Lk = int(P[p])
        CHk = max(1, min(TARGET_C // max(Lk, 1), nz - p))
        Lk = int(P[p:p + CHk].max())
        while CHk > 1 and CHk * Lk > MAX_C:
            CHk -= 1
            Lk = int(P[p:p + CHk].max())
        chunks.append((p, CHk, Lk, coloff))
        coloff += (CHk * Lk + 3) // 4 * 4
        p += CHk
    Ntot = coloff

    colstart = np.zeros(nz, dtype=np.int64)
    for (p0, CHk, Lk, off) in chunks:
        for r in range(CHk):
            colstart[p0 + r] = off + r * Lk

    for c in range(N_CORES):
        lane_rays = lane_rays_all[c]
        idxs = np.zeros((N_LANES, Ntot), dtype=np.int16)
        wts = np.zeros((N_LANES, Ntot, CSL), dtype=np.float16)
        raymap = []
        for pl in range(N_LANES):
            A, D, lens = lane_rays[pl]
            for pos in range(len(A)):
                a, dd, n4 = A[pos], D[pos], lens[pos]
                if n4 == 0:
                    continue
                cells, W = rays[a][dd]
                n = len(cells)
                col = colstart[pos]
                idxs[pl, col:col + n] = cells.astype(np.int16)
                # W [8w, 16slot, ncell] -> [ncell, 128]
                wts[pl, col:col + n, :] = (
                    W.reshape(CSL, n).T.astype(np.float16))
            raymap.append((A, D))

        # quad dictionary: distinct 4-cell groups across the whole stream;
        # idx stream entry i = qc*128 + lane -> (quad-col, lane) ordering.
        qcols = idxs.reshape(N_LANES, Ntot // 4, 4)      # [lane, qc, 4]
        flat = qcols.transpose(1, 0, 2).reshape(-1, 4)   # [(qc, lane), 4]
        uq, inv = np.unique(flat, axis=0, return_inverse=True)
        nq = len(uq)
        assert nq < 32000, f"quad dict overflow: {nq}"
        qidflat = inv.astype(np.int16)                   # [Ntot/4*128]
        idxw = np.zeros((128, Ntot * 128 // 4 // 16), dtype=np.int16)
        blk = qidflat.reshape(-1, 16).T
        for g in range(8):
            idxw[16 * g:16 * g + 16] = blk
        w01flat = wts.reshape(N_LANES, Ntot * CSL)
        cores.append(dict(idxw=idxw, w01flat=w01flat, raymap=raymap,
                          quads=uq.astype(np.int64), nq=nq))

    NQPAD = max(st["nq"] for st in cores)
    plan = dict(chunks=chunks, NR=NR, Ntot=Ntot, cores=cores, NQPAD=NQPAD)
    _plan_cache["plan"] = plan
    return plan


def _build_gtbl(x):
    """x [2,256,256] -> gtbl [NUM_ELEMS, 256] f16 HBM gather table.

    Row (ori*592 + u*37 + xq), col b*128 + w*16 + h*8 + j =
    img_ori[b, 16u + 8h + w, 7*xq + j] (zero-padded outside).
    """
    out = np.zeros((2, N_U, N_XQ, 2, 8, 2, 8), dtype=np.float16)
    for ori, im in enumerate((x, x.transpose(0, 2, 1))):
        img = np.zeros((2, 16 * N_U, XW * N_XQ + 1), dtype=np.float32)
        img[:, :VOL, :VOL] = im
        # [b, u, h, w, xq, j]
        v = img[:, :, :XW * N_XQ].reshape(2, N_U, 2, 8, N_XQ, XW)
        # j spans 8 px: 7*xq .. 7*xq+7 -> last col of next window start
        nxt = img[:, :, 7::7].reshape(2, N_U, 2, 8, N_XQ)
        out[ori, :, :, :, :, :, :XW] = v.transpose(1, 4, 0, 3, 2, 5)
        out[ori, :, :, :, :, :, XW] = nxt.transpose(1, 4, 0, 3, 2)
    return out.reshape(NUM_ELEMS, ESL)


def _build_gtbl4(gtbl, quads, nqpad):
    """Quad dictionary table: row q = concat of gtbl rows quads[q]."""
    gt4 = np.zeros((nqpad, 4 * ESL), dtype=np.float16)
    gt4[:len(quads)] = gtbl[quads].reshape(len(quads), 4 * ESL)
    return gt4


def _radon_kernel(tc, outs, ins, *, chunks, NR, Ntot):
    ctx = ExitStack()
    with ctx:
        nc = tc.nc
        gtbl_d, idx_d, w01_d = ins
        acc_d = outs[0]

        const_pool = ctx.enter_context(tc.tile_pool(name="const", bufs=1))
        g_pool = ctx.enter_context(tc.tile_pool(name="g", bufs=4))
        w_pool = ctx.enter_context(tc.tile_pool(name="w", bufs=2))
        t_pool = ctx.enter_context(tc.tile_pool(name="t", bufs=1))
        acc_pool = ctx.enter_context(tc.tile_pool(name="acc", bufs=1))

        idx_sb = const_pool.tile([128, Ntot * 2], I16)
        nc.sync.dma_start(idx_sb[:], idx_d[:])

        acc = acc_pool.tile([128, 2 * NR], F32)
        nc.vector.memset(acc[:], 0.0)

        qn = 0
        for (p0, CHk, Lk, off) in chunks:
            C = (CHk * Lk + 3) // 4 * 4
            gt = g_pool.tile([128, ESL * C], F16, tag="g")
            # quad descriptors (2KB): 4 cells per index. The SWDGE ring caps
            # a single gather at ~1024 descriptors; <=16-column sub-gathers
            # (512 descs) round-robined over the 4 SWDGE queues pipeline
            # more smoothly.
            for c0 in range(0, C, 16):
                cw = min(16, C - c0)
                nc.gpsimd.dma_gather(
                    gt[:, ESL * c0:ESL * (c0 + cw)]
                    .rearrange("p (c e) -> p c e", e=4 * ESL),
                    gtbl_d[:],
                    idx_sb[:, (off + c0) * 2:(off + c0 + cw) * 2],
                    num_idxs=128 * cw // 4,
                    num_idxs_reg=128 * cw // 4,
                    elem_size=4 * ESL,
                    queue_num=qn,
                )
                qn = (qn + 1) % 4
            wt = w_pool.tile([128, CSL * C], F16, tag="w")
            nc.sync.dma_start(
                wt[:], w01_d[:, off * CSL:(off + C) * CSL])

            # prod [0, 256C); tree levels packed at [256C, 510C). Keeping
            # the scratch OUT of gt ends gt's lifetime at the muls, so the
            # next gathers can run 3 chunks ahead of the DVE.
            st = t_pool.tile([128, 510 * C], F16, tag="t")
            g3 = gt[:].rearrange("p (c e) -> p c e", e=ESL)
            w3 = wt[:].rearrange("p (c e) -> p c e", e=CSL)
            pr = st[:, 0:ESL * C].rearrange("p (c e) -> p c e", e=ESL)
            nc.vector.tensor_mul(pr[:, :, 0:CSL], g3[:, :, 0:CSL], w3)
            nc.vector.tensor_mul(pr[:, :, CSL:ESL], g3[:, :, CSL:ESL], w3)
            # pairwise add tree over the 128 slots of each (cell, batch)
            # half-cells: 2C blocks of 128 -> 64 -> ... -> 1
            src = st[:, 0:ESL * C].rearrange("p (c e) -> p c e", e=CSL)
            base = ESL * C
            n = CSL // 2
            while n >= 1:
                dst = st[:, base:base + 2 * C * n].rearrange(
                    "p (c e) -> p c e", e=n)
                nc.vector.tensor_add(dst, src[:, :, 0:n], src[:, :, n:2 * n])
                src = dst
                base += 2 * C * n
                n //= 2
            # src is [p, 2C, 1]: per (cell, batch) sums at stride 2 per cell.
            # fold each ray's Lk cells: in [p, CHk, b(2), Lk], out [p, CHk, 2]
            nc.vector.tensor_reduce(
                acc[:, 2 * p0:2 * (p0 + CHk)]
                .rearrange("p (r b) -> p r b", b=2),
                st[:, base - 2 * C:base - 2 * C + 2 * CHk * Lk]
                .rearrange("p (r l b) -> p r b l", b=2, l=Lk),
                axis=mybir.AxisListType.X,
                op=mybir.AluOpType.add,
            )
        nc.sync.dma_start(acc_d[:], acc[:])


def _compile(plan):
    key = "nc9"
    if key in _compile_cache:
        return _compile_cache[key]
    Ntot, NR = plan["Ntot"], plan["NR"]
    nc = bacc.Bacc("TRN2", target_bir_lowering=False, debug=False,
                   enable_asserts=False, num_devices=N_CORES,
                   num_swdge_queues=4)
    gtbl_d = nc.dram_tensor("gtbl4", [plan["NQPAD"], 4 * ESL], F16,
                            kind="ExternalInput").ap()
    idx_d = nc.dram_tensor("idxw", [128, Ntot * 2], I16,
                           kind="ExternalInput").ap()
    w01_d = nc.dram_tensor("w01f", [128, Ntot * CSL], F16,
                           kind="ExternalInput").ap()
    acc_d = nc.dram_tensor("acc", [128, 2 * NR], F32,
                           kind="ExternalOutput").ap()
    with tile.TileContext(nc) as tc:
        _radon_kernel(tc, [acc_d], [gtbl_d, idx_d, w01_d],
                      chunks=plan["chunks"], NR=plan["NR"], Ntot=plan["Ntot"])
    nc.compile()
    _compile_cache[key] = nc
    return nc


def kernel(x):
    """x [2,256,256,1] f32 -> sinogram [2,180,363,1] f32."""
    x = np.asarray(x, dtype=np.float32)
    plan = _build_plan()
    gtbl = _build_gtbl(x[:, :, :, 0])
    nc = _compile(plan)
    in_maps = []
    for c in range(N_CORES):
        st = plan["cores"][c]
        gt4 = _build_gtbl4(gtbl, st["quads"], plan["NQPAD"])
        in_maps.append(dict(gtbl4=gt4, idxw=st["idxw"], w01f=st["w01flat"]))
    res = bass_utils.run_bass_kernel_spmd(nc, in_maps, core_ids=list(range(N_CORES)))
    NR = plan["NR"]
    sino = np.zeros((2, N_ANGLES, N_DET), dtype=np.float32)
    for c in range(N_CORES):
        accv = res.results[c]["acc"]
        for pl in range(N_LANES):
            A, D = plan["cores"][c]["raymap"][pl]
            n = min(len(A), NR)   # positions >= NR are zero-cell rays
            for b in range(2):
                sino[b, A[:n], D[:n]] = accv[pl, 2 * np.arange(n) + b]
    return sino[..., None]


if __name__ == "__main__":
    import time
    x = np.load("/tmp/x.npy")
    t0 = time.time()
    out = kernel(x)
    print("kernel() wall time:", time.time() - t0)
    exp = np.load("/tmp/expected_np.npy")
    rel = np.linalg.norm((out - exp).ravel()) / np.linalg.norm(exp.ravel())
    print("rel l2 vs numpy ref:", rel)


# revision 24
# speedup vs baseline: 1.0523x; 1.0523x over previous
"""Trainium2 Bass kernel v9: 2D parallel-beam forward projection (Radon).

Input:  x [2, 256, 256, 1] float32
Output: sinogram [2, 180, 363, 1] float32

Strategy (8 NeuronCores, SPMD), v9 = DMA-descriptor gather:
  - Angles interleaved across cores (core c: angles c::8). Per core, the
    ~8167 rays are dealt (longest-first round-robin) onto 128 partition
    lanes; each lane owns whole rays.
  - Cell = (16-row window u, 7-px x-window xq) of the (possibly
    transposed) image, as in v7: per-angle transpose trick keeps the
    ray's y-step >= 0.707, so a ray has ~20 cells.
  - Pixels: one HBM table gtbl[1184, 256] f16; row (ori, u, xq) holds
    the cell's 2 batches x 8 w-rows x (2h x 8j) pixels. nc.gpsimd.
    dma_gather streams cells: one 512B DMA descriptor per cell, index
    i -> partition i%128, so each lane receives its own cells in order.
    Descriptor generation is ~0.34ns/cell on GPSIMD and the copies run
    on the 16 DMA engines - the Q7 ap_gather bottleneck (34ns/idx) of
    v7/v8 is gone.
  - Weights (geometry-only, cached across calls): 128 f16 per cell
    (w x 2h x 8j), applied to both batches by two DVE muls.
  - DVE per chunk: 2 muls (fp16 2x), 7-level pairwise add tree (fp16
    2x) folding 128 -> 1 per (cell, batch), then one segmented
    tensor_reduce folding each ray's Lk cells into acc[128, 2*NR].
"""
import os
import sys
from contextlib import ExitStack

import numpy as np

for p in ("/opt/trn_rl_repo", "/root/.axon_site/_ro/trn_rl_repo"):
    if os.path.isdir(p) and p not in sys.path:
        sys.path.insert(0, p)

import concourse.bass as bass  # noqa: E402,F401
import concourse.bacc as bacc  # noqa: E402
import concourse.mybir as mybir  # noqa: E402
import concourse.tile as tile  # noqa: E402
from concourse import bass_utils  # noqa: E402

F32 = mybir.dt.float32
F16 = mybir.dt.float16
I16 = mybir.dt.int16

# ---- geometry constants (mirror of the reference) ----
VOL = 256
N_ANGLES = 180
N_DET = 363
N_SAMPLES = 363
CEN = (VOL - 1) / 2.0
DCEN = (N_DET - 1) / 2.0
SCEN = (N_SAMPLES - 1) / 2.0

N_U = 16          # 16-row windows
XW = 7            # x-slots covered per cell (entry spans XW+1 pixels)
N_XQ = 37         # ceil(257/7)
HALF = N_U * N_XQ             # 592 entries per orientation
NUM_ELEMS = 2 * HALF          # 1184 table rows
CSL = 128                     # weight slots per cell (8w x 2h x 8j)
ESL = 256                     # table row length in f16 (2 batches x CSL)

N_CORES = 8
N_LANES = 128
TARGET_C = 64    # cell-columns per chunk (<= MAX_C)
MAX_C = 64

_plan_cache = {}
_compile_cache = {}


def _f32(v):
    return np.float32(v)


def _ray_cells(k):
    """Merged cell decomposition for angle index k.

    Returns a list over detectors d of (cellidx[int], W[8, 16, ncell]); W
    slot 8*h + j (h = y-half) weights pixel img[16u + 8h + (w), 7*xq+j].
    """
    theta = _f32(k) * _f32(np.pi / N_ANGLES)
    c = np.cos(theta, dtype=np.float32)
    s = np.sin(theta, dtype=np.float32)
    flip = abs(s) > abs(c)
    ce, se = (s, c) if flip else (c, s)
    d = np.arange(N_DET, dtype=np.float32)[:, None] - _f32(DCEN)
    t = np.arange(N_SAMPLES, dtype=np.float32)[None, :] - _f32(SCEN)
    fx = (ce * d - se * t + _f32(CEN)).astype(np.float32)
    fy = (se * d + ce * t + _f32(CEN)).astype(np.float32)
    x0 = np.floor(fx).astype(np.int64)
    y0 = np.floor(fy).astype(np.int64)
    wx = (fx - x0).astype(np.float64)
    wy = (fy - y0).astype(np.float64)

    x_dead = (x0 < -1) | (x0 > 255)
    xslot = np.clip(x0, 0, 256)
    a0 = np.where((x0 >= 0) & (x0 < VOL), 1.0 - wx, 0.0)
    a1 = np.where((x0 + 1 >= 0) & (x0 + 1 < VOL), wx, 0.0)
    neg1 = x0 == -1
    a0 = np.where(neg1, wx, a0)
    a1 = np.where(neg1, 0.0, a1)

    lo = (~x_dead) & (y0 >= 0) & (y0 < VOL)
    hi = (~x_dead) & (y0 + 1 >= 0) & (y0 + 1 < VOL)

    D_idx = np.broadcast_to(np.arange(N_DET)[:, None], fx.shape)
    d_all = np.concatenate([D_idx[lo], D_idx[hi]])
    y_all = np.concatenate([y0[lo], y0[hi] + 1])
    tw_all = np.concatenate([(1.0 - wy)[lo], wy[hi]])
    a0_all = np.concatenate([a0[lo], a0[hi]])
    a1_all = np.concatenate([a1[lo], a1[hi]])
    xs_all = np.concatenate([xslot[lo], xslot[hi]])

    base = HALF if flip else 0
    u = y_all >> 4
    w = y_all & 7
    h = (y_all >> 3) & 1
    xq = xs_all // XW
    j0 = xs_all - xq * XW
    gkey = (d_all * N_U + u) * N_XQ + xq
    cells, inv = np.unique(gkey, return_inverse=True)
    ncell = len(cells)
    W = np.zeros((8, 16, ncell))
    np.add.at(W, (w, 8 * h + j0, inv), tw_all * a0_all)
    np.add.at(W, (w, 8 * h + j0 + 1, inv), tw_all * a1_all)
    cell_d = cells // (N_U * N_XQ)
    cell_e = cells % (N_U * N_XQ) + base
    bounds = np.searchsorted(cell_d, np.arange(N_DET + 1))
    out = []
    for dd in range(N_DET):
        i0, i1 = bounds[dd], bounds[dd + 1]
        out.append((cell_e[i0:i1], W[:, :, i0:i1]))
    return out


def _build_plan():
    """Geometry-only precompute shared across calls (input-independent)."""
    if "plan" in _plan_cache:
        return _plan_cache["plan"]
    rays = {}
    cnts = np.zeros((N_ANGLES, N_DET), dtype=np.int64)
    for k in range(N_ANGLES):
        rc = _ray_cells(k)
        rays[k] = rc
        cnts[k] = [len(r[0]) for r in rc]

    cores = []
    NRmax = 0
    lane_rays_all = []
    for c in range(N_CORES):
        ids = list(range(c, N_ANGLES, N_CORES))
        A, D = np.meshgrid(ids, np.arange(N_DET), indexing="ij")
        A, D = A.ravel(), D.ravel()
        lens = cnts[A, D]
        o = np.argsort(-lens, kind="stable")
        A, D, lens = A[o], D[o], lens[o]
        # snake-deal onto lanes to even out the sorted-length profile
        nround = (len(A) + N_LANES - 1) // N_LANES
        lane_idx = [[] for _ in range(N_LANES)]
        for r in range(nround):
            seg = range(r * N_LANES, min((r + 1) * N_LANES, len(A)))
            for i, gi in enumerate(seg):
                pl = i if r % 2 == 0 else (len(seg) - 1 - i)
                lane_idx[pl].append(gi)
        lane_rays = [(A[ix], D[ix], lens[ix])
                     for ix in (np.array(ix_, dtype=np.int64)
                                for ix_ in lane_idx)]
        lane_rays_all.append(lane_rays)
        NRmax = max(NRmax, max(len(a) for a, _, _ in lane_rays))

    # global sorted length profile (max across all lanes of all cores)
    P = np.zeros(NRmax, dtype=np.int64)
    for lane_rays in lane_rays_all:
        for A, D, lens in lane_rays:
            P[:len(lens)] = np.maximum(P[:len(lens)], lens)
    nz = int(np.count_nonzero(P))
    NR = nz

    # chunk schedule over ray positions: chunk = CHk rays x Lk cells.
    # Chunk column extents are padded to multiples of 4 so quad descriptors
    # align; rays themselves stay exact (quads may span ray boundaries).
    chunks = []   # (p0, CHk, Lk, coloff)
    coloff = 0
    p = 0
    while p < nz:
        Lk = int(P[p])
        CHk = max(1, min(TARGET_C // max(Lk, 1), nz - p))
        Lk = int(P[p:p + CHk].max())
        while CHk > 1 and CHk * Lk > MAX_C:
            CHk -= 1
            Lk = int(P[p:p + CHk].max())
        chunks.append((p, CHk, Lk, coloff))
        coloff += (CHk * Lk + 3) // 4 * 4
        p += CHk
    Ntot = coloff

    colstart = np.zeros(nz, dtype=np.int64)
    for (p0, CHk, Lk, off) in chunks:
        for r in range(CHk):
            colstart[p0 + r] = off + r * Lk

    for c in range(N_CORES):
        lane_rays = lane_rays_all[c]
        idxs = np.zeros((N_LANES, Ntot), dtype=np.int16)
        wts = np.zeros((N_LANES, Ntot, CSL), dtype=np.float16)
        raymap = []
        for pl in range(N_LANES):
            A, D, lens = lane_rays[pl]
            for pos in range(len(A)):
                a, dd, n4 = A[pos], D[pos], lens[pos]
                if n4 == 0:
                    continue
                cells, W = rays[a][dd]
                n = len(cells)
                col = colstart[pos]
                idxs[pl, col:col + n] = cells.astype(np.int16)
                # W [8w, 16slot, ncell] -> [ncell, 128]
                wts[pl, col:col + n, :] = (
                    W.reshape(CSL, n).T.astype(np.float16))
            raymap.append((A, D))

        # quad dictionary: distinct 4-cell groups across the whole stream;
        # idx stream entry i = qc*128 + lane -> (quad-col, lane) ordering.
        qcols = idxs.reshape(N_LANES, Ntot // 4, 4)      # [lane, qc, 4]
        flat = qcols.transpose(1, 0, 2).reshape(-1, 4)   # [(qc, lane), 4]
        uq, inv = np.unique(flat, axis=0, return_inverse=True)
        nq = len(uq)
        assert nq < 32000, f"quad dict overflow: {nq}"
        qidflat = inv.astype(np.int16)                   # [Ntot/4*128]
        idxw = np.zeros((128, Ntot * 128 // 4 // 16), dtype=np.int16)
        blk = qidflat.reshape(-1, 16).T
        for g in range(8):
            idxw[16 * g:16 * g + 16] = blk
        w01flat = wts.reshape(N_LANES, Ntot * CSL)
        cores.append(dict(idxw=idxw, w01flat=w01flat, raymap=raymap,
                          quads=uq.astype(np.int64), nq=nq))

    NQPAD = max(st["nq"] for st in cores)
    plan = dict(chunks=chunks, NR=NR, Ntot=Ntot, cores=cores, NQPAD=NQPAD)
    _plan_cache["plan"] = plan
    return plan


def _build_gtbl(x):
    """x [2,256,256] -> gtbl [NUM_ELEMS, 256] f16 HBM gather table.

    Row (ori*592 + u*37 + xq), col b*128 + w*16 + h*8 + j =
    img_ori[b, 16u + 8h + w, 7*xq + j] (zero-padded outside).
    """
    out = np.zeros((2, N_U, N_XQ, 2, 8, 2, 8), dtype=np.float16)
    for ori, im in enumerate((x, x.transpose(0, 2, 1))):
        img = np.zeros((2, 16 * N_U, XW * N_XQ + 1), dtype=np.float32)
        img[:, :VOL, :VOL] = im
        # [b, u, h, w, xq, j]
        v = img[:, :, :XW * N_XQ].reshape(2, N_U, 2, 8, N_XQ, XW)
        # j spans 8 px: 7*xq .. 7*xq+7 -> last col of next window start
        nxt = img[:, :, 7::7].reshape(2, N_U, 2, 8, N_XQ)
        out[ori, :, :, :, :, :, :XW] = v.transpose(1, 4, 0, 3, 2, 5)
        out[ori, :, :, :, :, :, XW] = nxt.transpose(1, 4, 0, 3, 2)
    return out.reshape(NUM_ELEMS, ESL)


def _build_gtbl4(gtbl, quads, nqpad):
    """Quad dictionary table: row q = concat of gtbl rows quads[q]."""
    gt4 = np.zeros((nqpad, 4 * ESL), dtype=np.float16)
    gt4[:len(quads)] = gtbl[quads].reshape(len(quads), 4 * ESL)
    return gt4


def _radon_kernel(tc, outs, ins, *, chunks, NR, Ntot):
    ctx = ExitStack()
    with ctx:
        nc = tc.nc
        gtbl_d, idx_d, w01_d = ins
        acc_d = outs[0]

        const_pool = ctx.enter_context(tc.tile_pool(name="const", bufs=1))
        g_pool = ctx.enter_context(tc.tile_pool(name="g", bufs=3))
        w_pool = ctx.enter_context(tc.tile_pool(name="w", bufs=2))
        t_pool = ctx.enter_context(tc.tile_pool(name="t", bufs=1))
        acc_pool = ctx.enter_context(tc.tile_pool(name="acc", bufs=1))

        idx_sb = const_pool.tile([128, Ntot * 2], I16)
        nc.sync.dma_start(idx_sb[:], idx_d[:])

        acc = acc_pool.tile([128, 2 * NR], F32)
        nc.vector.memset(acc[:], 0.0)

        qn = 0
        for (p0, CHk, Lk, off) in chunks:
            C = (CHk * Lk + 3) // 4 * 4
            gt = g_pool.tile([128, ESL * C], F16, tag="g")
            # quad descriptors (2KB): 4 cells per index. The SWDGE ring caps
            # a single gather at ~1024 descriptors; <=16-column sub-gathers
            # (512 descs) round-robined over the 4 SWDGE queues pipeline
            # more smoothly.
            for c0 in range(0, C, 16):
                cw = min(16, C - c0)
                nc.gpsimd.dma_gather(
                    gt[:, ESL * c0:ESL * (c0 + cw)]
                    .rearrange("p (c e) -> p c e", e=4 * ESL),
                    gtbl_d[:],
                    idx_sb[:, (off + c0) * 2:(off + c0 + cw) * 2],
                    num_idxs=128 * cw // 4,
                    num_idxs_reg=128 * cw // 4,
                    elem_size=4 * ESL,
                    queue_num=qn,
                )
                qn = (qn + 1) % 4
            wt = w_pool.tile([128, CSL * C], F16, tag="w")
            nc.sync.dma_start(
                wt[:], w01_d[:, off * CSL:(off + C) * CSL])

            # prod [0, 256C); tree levels packed at [256C, 510C). Keeping
            # the scratch OUT of gt ends gt's lifetime at the muls, so the
            # next gathers can run 3 chunks ahead of the DVE.
            st = t_pool.tile([128, 510 * C], F16, tag="t")
            g3 = gt[:].rearrange("p (c e) -> p c e", e=ESL)
            w3 = wt[:].rearrange("p (c e) -> p c e", e=CSL)
            pr = st[:, 0:ESL * C].rearrange("p (c e) -> p c e", e=ESL)
            nc.vector.tensor_mul(pr[:, :, 0:CSL], g3[:, :, 0:CSL], w3)
            nc.vector.tensor_mul(pr[:, :, CSL:ESL], g3[:, :, CSL:ESL], w3)
            # pairwise add tree over the 128 slots of each (cell, batch)
            # half-cells: 2C blocks of 128 -> 64 -> ... -> 1
            src = st[:, 0:ESL * C].rearrange("p (c e) -> p c e", e=CSL)
            base = ESL * C
            n = CSL // 2
            while n >= 1:
                dst = st[:, base:base + 2 * C * n].rearrange(
                    "p (c e) -> p c e", e=n)
                nc.vector.tensor_add(dst, src[:, :, 0:n], src[:, :, n:2 * n])
                src = dst
                base += 2 * C * n
                n //= 2
            # src is [p, 2C, 1]: per (cell, batch) sums at stride 2 per cell.
            # fold each ray's Lk cells: in [p, CHk, b(2), Lk], out [p, CHk, 2]
            nc.vector.tensor_reduce(
                acc[:, 2 * p0:2 * (p0 + CHk)]
                .rearrange("p (r b) -> p r b", b=2),
                st[:, base - 2 * C:base - 2 * C + 2 * CHk * Lk]
                .rearrange("p (r l b) -> p r b l", b=2, l=Lk),
                axis=mybir.AxisListType.X,
                op=mybir.AluOpType.add,
            )
        nc.sync.dma_start(acc_d[:], acc[:])


def _compile(plan):
    key = "nc9"
    if key in _compile_cache:
        return _compile_cache[key]
    Ntot, NR = plan["Ntot"], plan["NR"]
    nc = bacc.Bacc("TRN2", target_bir_lowering=False, debug=False,
                   enable_asserts=False, num_devices=N_CORES,
                   num_swdge_queues=4)
    gtbl_d = nc.dram_tensor("gtbl4", [plan["NQPAD"], 4 * ESL], F16,
                            kind="ExternalInput").ap()
    idx_d = nc.dram_tensor("idxw", [128, Ntot * 2], I16,
                           kind="ExternalInput").ap()
    w01_d = nc.dram_tensor("w01f", [128, Ntot * CSL], F16,
                           kind="ExternalInput").ap()
    acc_d = nc.dram_tensor("acc", [128, 2 * NR], F32,
                           kind="ExternalOutput").ap()
    with tile.TileContext(nc) as tc:
        _radon_kernel(tc, [acc_d], [gtbl_d, idx_d, w01_d],
                      chunks=plan["chunks"], NR=plan["NR"], Ntot=plan["Ntot"])
    nc.compile()
    _compile_cache[key] = nc
    return nc


def kernel(x):
    """x [2,256,256,1] f32 -> sinogram [2,180,363,1] f32."""
    x = np.asarray(x, dtype=np.float32)
    plan = _build_plan()
    gtbl = _build_gtbl(x[:, :, :, 0])
    nc = _compile(plan)
    in_maps = []
    for c in range(N_CORES):
        st = plan["cores"][c]
        gt4 = _build_gtbl4(gtbl, st["quads"], plan["NQPAD"])
        in_maps.append(dict(gtbl4=gt4, idxw=st["idxw"], w01f=st["w01flat"]))
    res = bass_utils.run_bass_kernel_spmd(nc, in_maps, core_ids=list(range(N_CORES)))
    NR = plan["NR"]
    sino = np.zeros((2, N_ANGLES, N_DET), dtype=np.float32)
    for c in range(N_CORES):
        accv = res.results[c]["acc"]
        for pl in range(N_LANES):
            A, D = plan["cores"][c]["raymap"][pl]
            n = min(len(A), NR)   # positions >= NR are zero-cell rays
            for b in range(2):
                sino[b, A[:n], D[:n]] = accv[pl, 2 * np.arange(n) + b]
    return sino[..., None]


if __name__ == "__main__":
    import time
    x = np.load("/tmp/x.npy")
    t0 = time.time()
    out = kernel(x)
    print("kernel() wall time:", time.time() - t0)
    exp = np.load("/tmp/expected_np.npy")
    rel = np.linalg.norm((out - exp).ravel()) / np.linalg.norm(exp.ravel())
    print("rel l2 vs numpy ref:", rel)
